# revision 24
# baseline (speedup 1.0000x reference)
"""Trainium2 Bass kernel for nn_IsoNSProject (Newton-Schulz polar projection).

reference:  A = U^T H U  (m = n-1, padded to n=2048)
            X0 = A/sigma_max; 10 Newton-Schulz steps X <- 0.5 X (3I - X^T X)
            H_out = e0 e0^T + U X10 U^T

Device algorithm (8-core SPMD, column-slab parallel, 6 NS steps fused 2x3):
  All NS iterates are polynomials of the Gram matrix C = A^T A, so they
  commute.  With g(x) = x(1.5-0.5x)^2 and phi(x) = (1-x/3)(1-g(x)/3):
      B_{2k+2} = g(g(B_{2k})),   Q <- phi(B_{2k}) Q,     B_0 = C/c^2
  and X6 = A Q with Q = (1.5^6/c) prod phi.  Six steps suffice: the scaled
  singular values start >= ~0.30 (c = sqrt(||C||_1) ~ 2.2 sigma_max) and
  reach 1 to fp32 accuracy in 6 NS steps (verified vs the 10-step
  reference: rel err ~1e-6 against tolerance 2e-2).

  Each double-step computes power slabs B^j s (j=2..9, 8 chained GEMMs of
  full^T @ slab with the full gathered matrix resident in SBUF as lhsT)
  and one AllGather of the new B slab; the phi(B) Q updates (4 GEMMs) and
  the DS1 Q-polynomial chain run in the shadow of the AllGathers.  Q is
  tracked without its identity component (phi's constant term is 1, so
  that component stays q0 = 1.5^6/c forever and is folded in at the end
  via q0*Uslab).

  Phase 1 avoids materializing A: with G = H U, G' = H^T U, w = G^T e0:
  C = G^T G - w w^T (since U U^T = I - e0 e0^T); the rank-1 term is one
  extra 1-partition matmul accumulation step.  The tail uses
  R = A Q = G'^T (U Q) and H-slab = e0 e0^T + U (R^T U^T-slab) (H is
  symmetric).  Collectives: AG_w, AG_[G|G'], AG_C, AllReduce(norm),
  AG_B2, AG_B4, AG_R -- vs 13 in the unfused 10-step version.
"""

import sys

for _p in ("/opt/trn_rl_repo", "/root/.axon_site/_ro/trn_rl_repo"):
    if _p not in sys.path:
        sys.path.insert(0, _p)

import numpy as np

import concourse.bass as bass
import concourse.tile as tile
from concourse import bacc
import concourse.mybir as mybir

N = 2048          # padded problem size (true m = 2047)
S = 256           # column-slab width per core
ET = N // 128     # 16 k-tiles
NCORES = 8
NSTEPS = 6

F32 = mybir.dt.float32
F32R = mybir.dt.float32r
ALU = mybir.AluOpType
AXT = mybir.AxisListType
ACT = mybir.ActivationFunctionType

# Two Newton-Schulz steps: an order-4 step X1 = X0 h7(B0) whose Gram
# recursion B1 = m7(B0) = B0 h7(B0)^2 (degree 7) is computed as power
# slabs, then an order-7 step X2 = X1 h13(B1) that needs no B2, so only
# the h13(B1) Q-factor (degree 6) is applied to the projected slab.
# From the c = sqrt(||C||_1) normalization the scaled singular values
# start >= 0.34 and reach 1 - 5e-3 (worst edge; ~1.5e-3 in F-norm)
# -- an order-28 composite, vs tolerance 2e-2.
H7 = [35.0 / 16, -35.0 / 16, 21.0 / 16, -5.0 / 16]
M7 = {j + 1: c for j, c in enumerate([
    4.78515625, -9.5703125, 10.52734375, -7.109375, 3.08984375,
    -0.8203125, 0.09765625])}
H13 = [2.9326171875, -5.865234375, 8.7978515625, -8.37890625,
       4.8876953125, -1.599609375, 0.2255859375]


def _build_nc():
    nc = bacc.Bacc(None, target_bir_lowering=False)

    HT_p = nc.declare_dram_parameter("HTm", [N, N], F32, isOutput=False)
    UT_p = nc.declare_dram_parameter("UTm", [N, N], F32, isOutput=False)
    Usl_p = nc.declare_dram_parameter("Uslab", [N, S], F32, isOutput=False)
    UTsl_p = nc.declare_dram_parameter("UTslab", [N, S], F32, isOutput=False)
    out_p = nc.declare_dram_parameter("Hslab", [N, S], F32, isOutput=True)

    RG = [list(range(NCORES))]

    with tile.TileContext(nc) as tc:
        with tc.tile_pool(name="dram", bufs=1, space="DRAM") as dram:
            bounceG = dram.tile([N, S], F32, name="bounceG")
            G_G = dram.tile([N * NCORES, S], F32, name="G_G")
            bounceW = dram.tile([1, S], F32, name="bounceW")
            G_W = dram.tile([1, NCORES * S], F32, name="G_W")
            bounceC = dram.tile([N, S], F32, name="bounceC")
            G_C = dram.tile([N * NCORES, S], F32, name="G_C")
            bounceB1 = dram.tile([N, S], F32, name="bounceB1")
            G_B2 = dram.tile([N * NCORES, S], F32, name="G_B2")
            bounceN = dram.tile([1, S], F32, name="bounceN")
            outN = dram.tile([1, S], F32, name="outN")

            def ag(in_t, out_t):
                nc.gpsimd.collective_compute(
                    "AllGather", ALU.bypass, replica_groups=RG,
                    ins=[in_t[:].opt()], outs=[out_t[:].opt()],
                )

            def param_block(p, col_off=0):
                def src(j):
                    return (p[:, col_off + S * j:col_off + S * (j + 1)]
                            .rearrange("(t p) d -> p t d", p=128).bitcast(F32R))
                return src

            def gathered_block(g, col_off=0):
                def src(j):
                    return (g[N * j:N * (j + 1), col_off:col_off + S]
                            .rearrange("(t p) d -> p t d", p=128).bitcast(F32R))
                return src

            body(tc, nc, locals())

    nc.compile()
    return nc


def body(tc, nc, T):
    HT_p, UT_p = T["HT_p"], T["UT_p"]
    Usl_p, UTsl_p, out_p = T["Usl_p"], T["UTsl_p"], T["out_p"]
    bounceG, G_G, bounceW, G_W = T["bounceG"], T["G_G"], T["bounceW"], T["G_W"]
    bounceC, G_C = T["bounceC"], T["G_C"]
    bounceB1, G_B2 = T["bounceB1"], T["G_B2"]
    bounceN, outN = T["bounceN"], T["outN"]
    ag, param_block, gathered_block = T["ag"], T["param_block"], T["gathered_block"]
    RG = [list(range(NCORES))]

    with (
        tc.tile_pool(name="lhs", bufs=1) as lhs,
        tc.tile_pool(name="lps", bufs=4, space="PSUM") as lps,
        tc.tile_pool(name="ltmp", bufs=2) as ltmp,
        tc.tile_pool(name="slab", bufs=1) as slab,
        tc.tile_pool(name="psc", bufs=1) as psc,
        tc.tile_pool(name="pscp", bufs=1, space="PSUM") as pscp,
    ):
        # lhsT block loads round-robin across three engine DMA queues so a
        # full-matrix load takes ~3 block-times instead of 8 (the cost of a
        # dma_start is charged to the issuing engine's queue).
        dma_engines = [nc.sync, nc.scalar, nc.gpsimd]

        def load_full(src, tagp):
            blks = []
            for j in range(NCORES):
                t = lhs.tile([128, ET, S], F32R, name=f"{tagp}{j}", tag=f"L{j}")
                dma_engines[j % len(dma_engines)].dma_start(t[:], src(j))
                blks.append(t)
            return blks

        def gemm(blocks, rhs_of_et, emit_out, nfree=S, extra_acc=None):
            """out[ct] = sum_et lhsT(et,ct).T @ rhs(et) (+ optional extra
            accumulation step issued with stop=True)."""
            for ct in range(ET):
                ps = lps.tile([128, nfree], F32, name="psr", tag="psr")
                j, h = ct // 2, ct % 2
                for et in range(ET):
                    nc.tensor.matmul(
                        ps[:, 0:nfree],
                        blocks[j][:, et, 128 * h:128 * (h + 1)],
                        rhs_of_et(et),
                        start=(et == 0),
                        stop=(et == ET - 1 and extra_acc is None),
                    )
                if extra_acc is not None:
                    extra_acc(ct, ps)
                emit_out(ct, ps)

        # four persistent slab slots, reused/retagged through the phases
        slotA = slab.tile([128, ET, S], F32R, name="slotA", tag="sA")
        slotB = slab.tile([128, ET, S], F32R, name="slotB", tag="sB")
        slotC = slab.tile([128, ET, S], F32R, name="slotC", tag="sC")
        slotD = slab.tile([128, ET, S], F32R, name="slotD", tag="sD")

        ones128 = psc.tile([128, 1], F32, name="ones128")
        nc.vector.memset(ones128[:], 1.0)
        ones_r = psc.tile([1, 128], F32, name="ones_r")
        nc.vector.memset(ones_r[:], 1.0)
        ws = psc.tile([1, S], F32, name="ws")
        w_neg = psc.tile([1, N], F32, name="w_neg")
        colsum = psc.tile([1, S], F32, name="colsum")
        colg = psc.tile([1, S], F32, name="colg")
        m11 = psc.tile([1, 1], F32, name="m11")

        # runtime scalars: rr^j = r^{2j} (j=1..7) at sc[:,j-1];
        # a_j = m7_j rr^j at sc[:,8+j]; q0 at sc[:,18];
        # e_j = h7_j r rr^j (j=1..3) at sc[:,18+j]; r at sc[:,23]
        sc = psc.tile([128, 24], F32, name="sc")

        def rrj(j):
            return sc[:, j - 1:j]

        def aj(j):
            return sc[:, 8 + j:9 + j]

        s_ap = sc[:, 18:19]

        def qj(j):
            return sc[:, 18 + j:19 + j]

        # ============ phase 1: G = H U (slotB), G' = H^T U, w ============
        # Uslab lives in slotD during phase 1 (freed by DS1's Bacc' writes)
        Uslab_sb = slotD
        nc.sync.dma_start(
            Uslab_sb[:],
            Usl_p.rearrange("(t p) d -> p t d", p=128).bitcast(F32R))

        ps_ws = pscp.tile([1, S], F32, name="ps_ws")
        HTb = load_full(param_block(HT_p), "HTb")

        def emit_g(ct, ps):
            nc.vector.tensor_copy(slotB[:, ct, :], ps[:, 0:S])
            nc.tensor.matmul(ps_ws[:], ones128[:],
                             slotB[:, ct, :].bitcast(F32),
                             start=(ct == 0), stop=(ct == ET - 1))
            nc.sync.dma_start(
                bounceG[128 * ct:128 * (ct + 1), :],
                slotB[:, ct, :].bitcast(F32))

        gemm(HTb, lambda et: Uslab_sb[:, et, :], emit_g)
        # ws = +colsum(G)/sqrt(n): own slab of w = G^T e0
        nc.vector.tensor_scalar_mul(ws[:], ps_ws[:], float(1.0 / np.sqrt(N)))
        nc.sync.dma_start(bounceW[:], ws[:])
        ag(bounceW, G_W)
        ag(bounceG, G_G)

        # ============ C slab (slotA) = G^T G - w w^T, norm ============
        nc.sync.dma_start(w_neg[:], G_W[:])
        nc.vector.tensor_scalar_mul(w_neg[:], w_neg[:], -1.0)

        Gb = load_full(gathered_block(G_G, 0), "Gb")
        ps_cs = pscp.tile([1, S], F32, name="ps_cs")

        def rank1_acc(ct, ps):
            nc.tensor.matmul(
                ps[:, 0:S], w_neg[0:1, 128 * ct:128 * (ct + 1)], ws[0:1, :],
                start=False, stop=True)

        def emit_c(ct, ps):
            nc.vector.tensor_copy(slotA[:, ct, :], ps[:, 0:S])
            ab = ltmp.tile([128, S], F32, name="absr", tag="t1")
            nc.vector.scalar_tensor_tensor(
                ab[:], slotA[:, ct, :].bitcast(F32), -1.0,
                slotA[:, ct, :].bitcast(F32),
                op0=ALU.mult, op1=ALU.max)
            nc.tensor.matmul(ps_cs[:], ones128[:], ab[:],
                             start=(ct == 0), stop=(ct == ET - 1))
            nc.sync.dma_start(
                bounceC[128 * ct:128 * (ct + 1), :],
                slotA[:, ct, :].bitcast(F32))

        gemm(Gb, lambda et: slotB[:, et, :], emit_c, extra_acc=rank1_acc)

        ag(bounceC, G_C)

        # ---- ||C||_1 bound -> runtime scalars ----
        nc.vector.tensor_copy(colsum[:], ps_cs[:])
        nc.sync.dma_start(bounceN[:], colsum[:])
        nc.gpsimd.collective_compute(
            "AllReduce", ALU.max, replica_groups=RG,
            ins=[bounceN[:].opt()], outs=[outN[:].opt()])
        nc.sync.dma_start(colg[:], outN[:])
        nc.vector.tensor_reduce(m11[:], colg[:], axis=AXT.X, op=ALU.max)
        ps_b = pscp.tile([128, 1], F32, name="ps_b")
        nc.tensor.matmul(ps_b[:], ones_r[:], m11[:], start=True, stop=True)

        # rr = 1/c^2 ; rr^j chain ; r = sqrt(rr) ; a_j = m7_j rr^j ;
        # q0 = h7_0 r ; e_j = h7_j r rr^j
        nc.vector.tensor_copy(rrj(1), ps_b[:])
        nc.vector.reciprocal(rrj(1), rrj(1))
        for j in range(2, 8):
            nc.vector.tensor_mul(rrj(j), rrj(j - 1), rrj(1))
        for j in range(2, 8):
            nc.vector.tensor_scalar_mul(aj(j), rrj(j), float(M7[j]))
        r_tmp = sc[:, 23:24]
        nc.scalar.activation(r_tmp, rrj(1), ACT.Sqrt)
        for j in range(1, 4):
            nc.vector.tensor_mul(qj(j), rrj(j), r_tmp)
            nc.vector.tensor_scalar_mul(qj(j), qj(j), float(H7[j]))
        nc.vector.tensor_scalar_mul(s_ap, r_tmp, float(H7[0]))
        # a_1 = m7_1 * rr
        nc.vector.tensor_scalar_mul(aj(1), rrj(1), float(M7[1]))

        # ============ DS1 (steps 0,1): powers of C ============
        # slots: A = Cs (-> Qacc), B = P-rot/B2s, C = P-rot/W-rot, D = Bacc'
        Cb = load_full(gathered_block(G_C, 0), "Cb")

        def power_chain(blocks, first_rhs, pslots, coeffs, acc, maxj=7):
            """P_{j+1} = M @ P_j for j=2..maxj; acc = sum_j coeffs[j]*P_j
            (incremental, initialized at the first present j).  coeffs[j]
            is an AP (runtime) or float immediate."""
            rhs = first_rhs
            init_j = min(coeffs)
            for j in range(2, maxj + 1):
                dst = pslots[(j - 2) % len(pslots)]

                def emit_p(ct, ps, j=j, dst=dst):
                    if j < maxj:
                        nc.vector.tensor_copy(dst[:, ct, :], ps[:, 0:S])
                    if j not in coeffs:
                        return
                    if j == init_j:
                        nc.vector.tensor_scalar_mul(
                            acc[:, ct, :], ps[:, 0:S], coeffs[j])
                    else:
                        nc.vector.scalar_tensor_tensor(
                            acc[:, ct, :], ps[:, 0:S], coeffs[j],
                            acc[:, ct, :].bitcast(F32),
                            op0=ALU.mult, op1=ALU.add)

                gemm(blocks, (lambda r: lambda et: r[:, et, :])(rhs), emit_p)
                if j < maxj:
                    rhs = dst

        # powers P2..P7 of C; Bacc' (slotD) = sum_{j=2..7} a_j C^j s
        power_chain(Cb, slotA, [slotB, slotC],
                    {j: aj(j) for j in range(2, 8)}, slotD, maxj=7)

        # bounce B2s = a1*Cs + Bacc' into slotB (P8's old slot), then AG
        for ct in range(ET):
            nc.vector.scalar_tensor_tensor(
                slotB[:, ct, :], slotA[:, ct, :].bitcast(F32), aj(1),
                slotD[:, ct, :].bitcast(F32), op0=ALU.mult, op1=ALU.add)
        nc.sync.dma_start(
            bounceB1[:].rearrange("(t p) d -> p t d", p=128),
            slotB[:].bitcast(F32))
        ag(bounceB1, G_B2)

        # ====== u-chain (shadow of AG_B1): z1' = Q_1 @ UTslab computed
        # directly from C-powers applied to the UTslab vector-slab:
        # z1' = q0*UTslab + sum_{j=1..4} e_j (C^j @ UTslab).
        # u_j -> rot(B, C); z1' accumulates in slotA (Cs is dead).
        UTslab2 = slotD
        nc.sync.dma_start(
            UTslab2[:],
            UTsl_p.rearrange("(t p) d -> p t d", p=128).bitcast(F32R))

        def emit_u1(ct, ps):
            nc.vector.tensor_copy(slotB[:, ct, :], ps[:, 0:S])
            nc.vector.scalar_tensor_tensor(
                slotA[:, ct, :], ps[:, 0:S], qj(1),
                slotA[:, ct, :].bitcast(F32), op0=ALU.mult, op1=ALU.add)

        # init A = q0 * UTslab first (DVE, before the u-chain)
        for ct in range(ET):
            nc.vector.tensor_scalar_mul(
                slotA[:, ct, :], UTslab2[:, ct, :].bitcast(F32), s_ap)

        gemm(Cb, lambda et: UTslab2[:, et, :], emit_u1)

        rhs = slotB
        for j in range(2, 4):
            dst = [slotC, slotB][j % 2]

            def emit_uj(ct, ps, j=j, dst=dst):
                if j < 3:
                    nc.vector.tensor_copy(dst[:, ct, :], ps[:, 0:S])
                nc.vector.scalar_tensor_tensor(
                    slotA[:, ct, :], ps[:, 0:S], qj(j),
                    slotA[:, ct, :].bitcast(F32), op0=ALU.mult, op1=ALU.add)

            gemm(Cb, (lambda r: lambda et: r[:, et, :])(rhs), emit_uj)
            if j < 3:
                rhs = dst

        # ====== v-chain: z1 = h13(B1) @ z1' (order-7 step applied to the
        # projected slab); v_1 = B1@z1', v_{j+1} = B1@v_j, and
        # z1 (slotB) = h13_0 z1' + sum_{j=1..6} h13_j v_j.
        B1b = load_full(gathered_block(G_B2, 0), "B1b")

        def emit_v1(ct, ps):
            nc.vector.tensor_copy(slotC[:, ct, :], ps[:, 0:S])
            nc.vector.tensor_scalar_mul(
                slotB[:, ct, :], slotA[:, ct, :].bitcast(F32), float(H13[0]))
            nc.vector.scalar_tensor_tensor(
                slotB[:, ct, :], ps[:, 0:S], float(H13[1]),
                slotB[:, ct, :].bitcast(F32), op0=ALU.mult, op1=ALU.add)

        gemm(B1b, lambda et: slotA[:, et, :], emit_v1)
        rhs = slotC
        for j in range(2, 7):
            dst = [slotD, slotC][j % 2]

            def emit_vj(ct, ps, j=j, dst=dst):
                if j < 6:
                    nc.vector.tensor_copy(dst[:, ct, :], ps[:, 0:S])
                nc.vector.scalar_tensor_tensor(
                    slotB[:, ct, :], ps[:, 0:S], float(H13[j]),
                    slotB[:, ct, :].bitcast(F32), op0=ALU.mult, op1=ALU.add)

            gemm(B1b, (lambda r: lambda et: r[:, et, :])(rhs), emit_vj)
            if j < 6:
                rhs = dst

        # t1 = U @ z1 -> slotC
        UTb = load_full(param_block(UT_p), "UTb")

        def emit_t1(ct, ps):
            nc.vector.tensor_copy(slotC[:, ct, :], ps[:, 0:S])

        gemm(UTb, lambda et: slotB[:, et, :], emit_t1)

        # t2 = H @ t1 -> slotA (with interleaved column sums).  Then
        # U U^T t2 = (I - e0 e0^T) t2, so
        # H-slab = e0 e0^T-slab + t2 - e0 (e0^T t2)
        #        = t2 + ones * (1 - colsum(t2))/n  (rank-1 row correction).
        HTb2 = load_full(param_block(HT_p), "HTb2")
        ps_t = pscp.tile([1, S], F32, name="ps_t")

        def emit_t2(ct, ps):
            nc.vector.tensor_copy(slotA[:, ct, :], ps[:, 0:S])
            nc.tensor.matmul(ps_t[:], ones128[:],
                             slotA[:, ct, :].bitcast(F32),
                             start=(ct == 0), stop=(ct == ET - 1))

        gemm(HTb2, lambda et: slotC[:, et, :], emit_t2)

        w2 = psc.tile([1, S], F32, name="w2")
        on1 = psc.tile([1, S], F32, name="on1")
        nc.vector.memset(on1[:], 1.0 / N)
        nc.vector.scalar_tensor_tensor(
            w2[:], ps_t[:], -1.0 / N, on1[:], op0=ALU.mult, op1=ALU.add)

        for ct in range(ET):
            ps2 = lps.tile([128, S], F32, name="psr2", tag="psr")
            nc.tensor.matmul(ps2[:], ones_r[0:1, :], w2[0:1, :],
                             start=True, stop=True)
            h1 = ltmp.tile([128, S], F32, name="h1", tag="t1")
            nc.vector.tensor_add(h1[:], slotA[:, ct, :].bitcast(F32), ps2[:])
            nc.sync.dma_start(out_p[128 * ct:128 * (ct + 1), :], h1[:])


_CACHED = {}


def _get_nc():
    if "nc" not in _CACHED:
        _CACHED["nc"] = _build_nc()
    return _CACHED["nc"]


def make_in_maps(H_raw, U):
    H_raw = np.ascontiguousarray(H_raw, np.float32)
    assert H_raw.shape == (N, N)
    Upad = np.zeros((N, N), np.float32)
    Upad[:, :U.shape[1]] = np.asarray(U, np.float32)
    HT = np.ascontiguousarray(H_raw.T)
    UT = np.ascontiguousarray(Upad.T)
    in_maps = []
    for i in range(NCORES):
        sl = slice(S * i, S * (i + 1))
        in_maps.append({
            "HTm": HT, "UTm": UT,
            "Uslab": np.ascontiguousarray(Upad[:, sl]),
            "UTslab": np.ascontiguousarray(UT[:, sl]),
        })
    return in_maps


def assemble(results):
    return np.ascontiguousarray(
        np.concatenate([results[i]["Hslab"] for i in range(NCORES)], axis=1),
        dtype=np.float32)


def kernel(H_raw, U):
    from concourse.bass_utils import run_bass_kernel_spmd
    nc = _get_nc()
    in_maps = make_in_maps(H_raw, U)
    res = run_bass_kernel_spmd(nc, in_maps, core_ids=list(range(NCORES)))
    return assemble(res.results)


if __name__ == "__main__":
    rng = np.random.default_rng(0)
    H_raw = (np.eye(N) + 0.1 / np.sqrt(N)
             * rng.standard_normal((N, N))).astype(np.float32)
    Uq, _ = np.linalg.qr(rng.standard_normal((N, N - 1)).astype(np.float32))
    out = kernel(H_raw, Uq.astype(np.float32))
    print("kernel output", out.shape, out.dtype)


# revision 25
# speedup vs baseline: 1.4174x; 1.4174x over previous
"""Trainium2 Bass kernel for nn_IsoNSProject (Newton-Schulz polar projection).

reference:  A = U^T H U  (m = n-1, padded to n=2048)
            X0 = A/sigma_max; 10 Newton-Schulz steps X <- 0.5 X (3I - X^T X)
            H_out = e0 e0^T + U X10 U^T

Device algorithm (8-core SPMD, column-slab tensor-parallel):
  All NS iterates commute (they are polynomials of C = A^T A), so the ten
  quadratic steps are replaced by an order-28 composite of two high-order
  NS steps: an order-4 step whose Gram recursion B1 = m7(B0) = B0 h7(B0)^2
  is computed as power slabs (6 chained GEMMs of resident-full^T @ slab),
  and an order-7 step that needs no successor Gram matrix, so only its
  Q-factor h13(B1) is applied -- directly to the projected slab
  z1' = Q1 @ U^T-slab, which itself is built from C-powers applied to
  U^T-slab (u-chain) in the shadow of B1's AllGather.  From the on-device
  bound c = sqrt(||C||_1) >= sigma_max(A), the scaled singular values
  start >= 0.34 and converge to 1 - ~5e-3 worst-edge (~1.8e-3 F-norm
  vs tolerance 2e-2).

  Phase 1 avoids materializing A: with G = H U and w = G^T e0,
  C = G^T G - w w^T (U U^T = I - e0 e0^T); the rank-1 term is one extra
  1-partition matmul accumulation step, and w comes from an 8KB
  AllGather of per-core column sums.  The tail exploits the projector
  again: out = t2 + ones (1 - colsum(t2))/n with t2 = H U z1, removing
  two full GEMMs.  Q's identity component q0 (tracked as a runtime
  scalar) is folded into z1' as q0 * U^T-slab.

  Collectives (5 total): AG_w (8KB), AG_G, AG_C, AllReduce(norm max),
  AG_B1 -- each 2MB-per-core slab gather; the u-chain (3 GEMMs) hides
  under AG_B1.  lhsT block loads are spread over the SP/ACT/Pool DMA
  queues; all matmuls run as float32r (full rate) with fp32 PSUM.
"""

import sys

for _p in ("/opt/trn_rl_repo", "/root/.axon_site/_ro/trn_rl_repo"):
    if _p not in sys.path:
        sys.path.insert(0, _p)

import numpy as np

import concourse.bass as bass
import concourse.tile as tile
from concourse import bacc
import concourse.mybir as mybir

N = 2048          # padded problem size (true m = 2047)
S = 256           # column-slab width per core
ET = N // 128     # 16 k-tiles
NCORES = 8

F32 = mybir.dt.float32
F32R = mybir.dt.float32r
ALU = mybir.AluOpType
AXT = mybir.AxisListType
ACT = mybir.ActivationFunctionType

# Two Newton-Schulz steps: an order-4 step X1 = X0 h7(B0) whose Gram
# recursion B1 = m7(B0) = B0 h7(B0)^2 (degree 7) is computed as power
# slabs, then an order-7 step X2 = X1 h13(B1) that needs no B2, so only
# the h13(B1) Q-factor (degree 6) is applied to the projected slab.
# From the c = sqrt(||C||_1) normalization the scaled singular values
# start >= 0.34 and reach 1 - 5e-3 (worst edge; ~1.5e-3 in F-norm)
# -- an order-28 composite, vs tolerance 2e-2.
H7 = [35.0 / 16, -35.0 / 16, 21.0 / 16, -5.0 / 16]
M7 = {j + 1: c for j, c in enumerate([
    4.78515625, -9.5703125, 10.52734375, -7.109375, 3.08984375,
    -0.8203125, 0.09765625])}
H13 = [2.9326171875, -5.865234375, 8.7978515625, -8.37890625,
       4.8876953125, -1.599609375, 0.2255859375]


def _build_nc():
    nc = bacc.Bacc(None, target_bir_lowering=False)

    HT_p = nc.declare_dram_parameter("HTm", [N, N], F32, isOutput=False)
    UT_p = nc.declare_dram_parameter("UTm", [N, N], F32, isOutput=False)
    Usl_p = nc.declare_dram_parameter("Uslab", [N, S], F32, isOutput=False)
    UTsl_p = nc.declare_dram_parameter("UTslab", [N, S], F32, isOutput=False)
    out_p = nc.declare_dram_parameter("Hslab", [N, S], F32, isOutput=True)

    RG = [list(range(NCORES))]

    with tile.TileContext(nc) as tc:
        with tc.tile_pool(name="dram", bufs=1, space="DRAM") as dram:
            bounceG = dram.tile([N, S], F32, name="bounceG")
            G_G = dram.tile([N * NCORES, S], F32, name="G_G")
            bounceW = dram.tile([1, S], F32, name="bounceW")
            G_W = dram.tile([1, NCORES * S], F32, name="G_W")
            bounceC = dram.tile([N, S], F32, name="bounceC")
            G_C = dram.tile([N * NCORES, S], F32, name="G_C")
            bounceB1 = dram.tile([N, S], F32, name="bounceB1")
            G_B2 = dram.tile([N * NCORES, S], F32, name="G_B2")
            bounceN = dram.tile([1, S], F32, name="bounceN")
            outN = dram.tile([1, S], F32, name="outN")

            def ag(in_t, out_t):
                nc.gpsimd.collective_compute(
                    "AllGather", ALU.bypass, replica_groups=RG,
                    ins=[in_t[:].opt()], outs=[out_t[:].opt()],
                )

            def param_block(p, col_off=0):
                def src(j):
                    return (p[:, col_off + S * j:col_off + S * (j + 1)]
                            .rearrange("(t p) d -> p t d", p=128).bitcast(F32R))
                return src

            def gathered_block(g, col_off=0):
                def src(j):
                    return (g[N * j:N * (j + 1), col_off:col_off + S]
                            .rearrange("(t p) d -> p t d", p=128).bitcast(F32R))
                return src

            body(tc, nc, locals())

    nc.compile()
    return nc


def body(tc, nc, T):
    HT_p, UT_p = T["HT_p"], T["UT_p"]
    Usl_p, UTsl_p, out_p = T["Usl_p"], T["UTsl_p"], T["out_p"]
    bounceG, G_G, bounceW, G_W = T["bounceG"], T["G_G"], T["bounceW"], T["G_W"]
    bounceC, G_C = T["bounceC"], T["G_C"]
    bounceB1, G_B2 = T["bounceB1"], T["G_B2"]
    bounceN, outN = T["bounceN"], T["outN"]
    ag, param_block, gathered_block = T["ag"], T["param_block"], T["gathered_block"]
    RG = [list(range(NCORES))]

    with (
        tc.tile_pool(name="lhs", bufs=1) as lhs,
        tc.tile_pool(name="lps", bufs=4, space="PSUM") as lps,
        tc.tile_pool(name="ltmp", bufs=2) as ltmp,
        tc.tile_pool(name="slab", bufs=1) as slab,
        tc.tile_pool(name="psc", bufs=1) as psc,
        tc.tile_pool(name="pscp", bufs=1, space="PSUM") as pscp,
    ):
        # lhsT block loads round-robin across three engine DMA queues so a
        # full-matrix load takes ~3 block-times instead of 8 (the cost of a
        # dma_start is charged to the issuing engine's queue).
        dma_engines = [nc.sync, nc.scalar, nc.gpsimd]

        def load_full(src, tagp):
            blks = []
            for j in range(NCORES):
                t = lhs.tile([128, ET, S], F32R, name=f"{tagp}{j}", tag=f"L{j}")
                dma_engines[j % len(dma_engines)].dma_start(t[:], src(j))
                blks.append(t)
            return blks

        def gemm(blocks, rhs_of_et, emit_out, nfree=S, extra_acc=None):
            """out[ct] = sum_et lhsT(et,ct).T @ rhs(et) (+ optional extra
            accumulation step issued with stop=True)."""
            for ct in range(ET):
                ps = lps.tile([128, nfree], F32, name="psr", tag="psr")
                j, h = ct // 2, ct % 2
                for et in range(ET):
                    nc.tensor.matmul(
                        ps[:, 0:nfree],
                        blocks[j][:, et, 128 * h:128 * (h + 1)],
                        rhs_of_et(et),
                        start=(et == 0),
                        stop=(et == ET - 1 and extra_acc is None),
                    )
                if extra_acc is not None:
                    extra_acc(ct, ps)
                emit_out(ct, ps)

        # four persistent slab slots, reused/retagged through the phases
        slotA = slab.tile([128, ET, S], F32R, name="slotA", tag="sA")
        slotB = slab.tile([128, ET, S], F32R, name="slotB", tag="sB")
        slotC = slab.tile([128, ET, S], F32R, name="slotC", tag="sC")
        slotD = slab.tile([128, ET, S], F32R, name="slotD", tag="sD")

        ones128 = psc.tile([128, 1], F32, name="ones128")
        nc.vector.memset(ones128[:], 1.0)
        ones_r = psc.tile([1, 128], F32, name="ones_r")
        nc.vector.memset(ones_r[:], 1.0)
        ws = psc.tile([1, S], F32, name="ws")
        w_neg = psc.tile([1, N], F32, name="w_neg")
        colsum = psc.tile([1, S], F32, name="colsum")
        colg = psc.tile([1, S], F32, name="colg")
        m11 = psc.tile([1, 1], F32, name="m11")

        # runtime scalars: rr^j = r^{2j} (j=1..7) at sc[:,j-1];
        # a_j = m7_j rr^j at sc[:,8+j]; q0 at sc[:,18];
        # e_j = h7_j r rr^j (j=1..3) at sc[:,18+j]; r at sc[:,23]
        sc = psc.tile([128, 24], F32, name="sc")

        def rrj(j):
            return sc[:, j - 1:j]

        def aj(j):
            return sc[:, 8 + j:9 + j]

        s_ap = sc[:, 18:19]

        def qj(j):
            return sc[:, 18 + j:19 + j]

        # ============ phase 1: G = H U (slotB), G' = H^T U, w ============
        # Uslab lives in slotD during phase 1 (freed by DS1's Bacc' writes)
        Uslab_sb = slotD
        nc.sync.dma_start(
            Uslab_sb[:],
            Usl_p.rearrange("(t p) d -> p t d", p=128).bitcast(F32R))

        ps_ws = pscp.tile([1, S], F32, name="ps_ws")
        HTb = load_full(param_block(HT_p), "HTb")

        def emit_g(ct, ps):
            nc.vector.tensor_copy(slotB[:, ct, :], ps[:, 0:S])
            nc.tensor.matmul(ps_ws[:], ones128[:],
                             slotB[:, ct, :].bitcast(F32),
                             start=(ct == 0), stop=(ct == ET - 1))
            nc.sync.dma_start(
                bounceG[128 * ct:128 * (ct + 1), :],
                slotB[:, ct, :].bitcast(F32))

        gemm(HTb, lambda et: Uslab_sb[:, et, :], emit_g)
        # ws = +colsum(G)/sqrt(n): own slab of w = G^T e0
        nc.vector.tensor_scalar_mul(ws[:], ps_ws[:], float(1.0 / np.sqrt(N)))
        nc.sync.dma_start(bounceW[:], ws[:])
        ag(bounceW, G_W)
        ag(bounceG, G_G)

        # ============ C slab (slotA) = G^T G - w w^T, norm ============
        nc.sync.dma_start(w_neg[:], G_W[:])
        nc.vector.tensor_scalar_mul(w_neg[:], w_neg[:], -1.0)

        Gb = load_full(gathered_block(G_G, 0), "Gb")
        ps_cs = pscp.tile([1, S], F32, name="ps_cs")

        def rank1_acc(ct, ps):
            nc.tensor.matmul(
                ps[:, 0:S], w_neg[0:1, 128 * ct:128 * (ct + 1)], ws[0:1, :],
                start=False, stop=True)

        def emit_c(ct, ps):
            nc.vector.tensor_copy(slotA[:, ct, :], ps[:, 0:S])
            ab = ltmp.tile([128, S], F32, name="absr", tag="t1")
            nc.vector.scalar_tensor_tensor(
                ab[:], slotA[:, ct, :].bitcast(F32), -1.0,
                slotA[:, ct, :].bitcast(F32),
                op0=ALU.mult, op1=ALU.max)
            nc.tensor.matmul(ps_cs[:], ones128[:], ab[:],
                             start=(ct == 0), stop=(ct == ET - 1))
            nc.sync.dma_start(
                bounceC[128 * ct:128 * (ct + 1), :],
                slotA[:, ct, :].bitcast(F32))

        gemm(Gb, lambda et: slotB[:, et, :], emit_c, extra_acc=rank1_acc)

        ag(bounceC, G_C)

        # ---- ||C||_1 bound -> runtime scalars ----
        nc.vector.tensor_copy(colsum[:], ps_cs[:])
        nc.sync.dma_start(bounceN[:], colsum[:])
        nc.gpsimd.collective_compute(
            "AllReduce", ALU.max, replica_groups=RG,
            ins=[bounceN[:].opt()], outs=[outN[:].opt()])
        nc.sync.dma_start(colg[:], outN[:])
        nc.vector.tensor_reduce(m11[:], colg[:], axis=AXT.X, op=ALU.max)
        ps_b = pscp.tile([128, 1], F32, name="ps_b")
        nc.tensor.matmul(ps_b[:], ones_r[:], m11[:], start=True, stop=True)

        # rr = 1/c^2 ; rr^j chain ; r = sqrt(rr) ; a_j = m7_j rr^j ;
        # q0 = h7_0 r ; e_j = h7_j r rr^j
        nc.vector.tensor_copy(rrj(1), ps_b[:])
        nc.vector.reciprocal(rrj(1), rrj(1))
        for j in range(2, 8):
            nc.vector.tensor_mul(rrj(j), rrj(j - 1), rrj(1))
        for j in range(2, 8):
            nc.vector.tensor_scalar_mul(aj(j), rrj(j), float(M7[j]))
        r_tmp = sc[:, 23:24]
        nc.scalar.activation(r_tmp, rrj(1), ACT.Sqrt)
        for j in range(1, 4):
            nc.vector.tensor_mul(qj(j), rrj(j), r_tmp)
            nc.vector.tensor_scalar_mul(qj(j), qj(j), float(H7[j]))
        nc.vector.tensor_scalar_mul(s_ap, r_tmp, float(H7[0]))
        # a_1 = m7_1 * rr
        nc.vector.tensor_scalar_mul(aj(1), rrj(1), float(M7[1]))

        # ============ DS1 (steps 0,1): powers of C ============
        # slots: A = Cs (-> Qacc), B = P-rot/B2s, C = P-rot/W-rot, D = Bacc'
        Cb = load_full(gathered_block(G_C, 0), "Cb")

        def power_chain(blocks, first_rhs, pslots, coeffs, acc, maxj=7):
            """P_{j+1} = M @ P_j for j=2..maxj; acc = sum_j coeffs[j]*P_j
            (incremental, initialized at the first present j).  coeffs[j]
            is an AP (runtime) or float immediate."""
            rhs = first_rhs
            init_j = min(coeffs)
            for j in range(2, maxj + 1):
                dst = pslots[(j - 2) % len(pslots)]

                def emit_p(ct, ps, j=j, dst=dst):
                    if j < maxj:
                        nc.vector.tensor_copy(dst[:, ct, :], ps[:, 0:S])
                    if j not in coeffs:
                        return
                    if j == init_j:
                        nc.vector.tensor_scalar_mul(
                            acc[:, ct, :], ps[:, 0:S], coeffs[j])
                    else:
                        nc.vector.scalar_tensor_tensor(
                            acc[:, ct, :], ps[:, 0:S], coeffs[j],
                            acc[:, ct, :].bitcast(F32),
                            op0=ALU.mult, op1=ALU.add)

                gemm(blocks, (lambda r: lambda et: r[:, et, :])(rhs), emit_p)
                if j < maxj:
                    rhs = dst

        # powers P2..P7 of C; Bacc' (slotD) = sum_{j=2..7} a_j C^j s
        power_chain(Cb, slotA, [slotB, slotC],
                    {j: aj(j) for j in range(2, 8)}, slotD, maxj=7)

        # bounce B2s = a1*Cs + Bacc' into slotB (P8's old slot), then AG
        for ct in range(ET):
            nc.vector.scalar_tensor_tensor(
                slotB[:, ct, :], slotA[:, ct, :].bitcast(F32), aj(1),
                slotD[:, ct, :].bitcast(F32), op0=ALU.mult, op1=ALU.add)
        nc.sync.dma_start(
            bounceB1[:].rearrange("(t p) d -> p t d", p=128),
            slotB[:].bitcast(F32))
        ag(bounceB1, G_B2)

        # ====== u-chain (shadow of AG_B1): z1' = Q_1 @ UTslab computed
        # directly from C-powers applied to the UTslab vector-slab:
        # z1' = q0*UTslab + sum_{j=1..4} e_j (C^j @ UTslab).
        # u_j -> rot(B, C); z1' accumulates in slotA (Cs is dead).
        UTslab2 = slotD
        nc.sync.dma_start(
            UTslab2[:],
            UTsl_p.rearrange("(t p) d -> p t d", p=128).bitcast(F32R))

        def emit_u1(ct, ps):
            nc.vector.tensor_copy(slotB[:, ct, :], ps[:, 0:S])
            nc.vector.scalar_tensor_tensor(
                slotA[:, ct, :], ps[:, 0:S], qj(1),
                slotA[:, ct, :].bitcast(F32), op0=ALU.mult, op1=ALU.add)

        # init A = q0 * UTslab first (DVE, before the u-chain)
        for ct in range(ET):
            nc.vector.tensor_scalar_mul(
                slotA[:, ct, :], UTslab2[:, ct, :].bitcast(F32), s_ap)

        gemm(Cb, lambda et: UTslab2[:, et, :], emit_u1)

        rhs = slotB
        for j in range(2, 4):
            dst = [slotC, slotB][j % 2]

            def emit_uj(ct, ps, j=j, dst=dst):
                if j < 3:
                    nc.vector.tensor_copy(dst[:, ct, :], ps[:, 0:S])
                nc.vector.scalar_tensor_tensor(
                    slotA[:, ct, :], ps[:, 0:S], qj(j),
                    slotA[:, ct, :].bitcast(F32), op0=ALU.mult, op1=ALU.add)

            gemm(Cb, (lambda r: lambda et: r[:, et, :])(rhs), emit_uj)
            if j < 3:
                rhs = dst

        # ====== v-chain: z1 = h13(B1) @ z1' (order-7 step applied to the
        # projected slab); v_1 = B1@z1', v_{j+1} = B1@v_j, and
        # z1 (slotB) = h13_0 z1' + sum_{j=1..6} h13_j v_j.
        B1b = load_full(gathered_block(G_B2, 0), "B1b")

        def emit_v1(ct, ps):
            nc.vector.tensor_copy(slotC[:, ct, :], ps[:, 0:S])
            nc.vector.tensor_scalar_mul(
                slotB[:, ct, :], slotA[:, ct, :].bitcast(F32), float(H13[0]))
            nc.vector.scalar_tensor_tensor(
                slotB[:, ct, :], ps[:, 0:S], float(H13[1]),
                slotB[:, ct, :].bitcast(F32), op0=ALU.mult, op1=ALU.add)

        gemm(B1b, lambda et: slotA[:, et, :], emit_v1)
        rhs = slotC
        for j in range(2, 7):
            dst = [slotD, slotC][j % 2]

            def emit_vj(ct, ps, j=j, dst=dst):
                if j < 6:
                    nc.vector.tensor_copy(dst[:, ct, :], ps[:, 0:S])
                nc.vector.scalar_tensor_tensor(
                    slotB[:, ct, :], ps[:, 0:S], float(H13[j]),
                    slotB[:, ct, :].bitcast(F32), op0=ALU.mult, op1=ALU.add)

            gemm(B1b, (lambda r: lambda et: r[:, et, :])(rhs), emit_vj)
            if j < 6:
                rhs = dst

        # t1 = U @ z1 -> slotC
        UTb = load_full(param_block(UT_p), "UTb")

        def emit_t1(ct, ps):
            nc.vector.tensor_copy(slotC[:, ct, :], ps[:, 0:S])

        gemm(UTb, lambda et: slotB[:, et, :], emit_t1)

        # t2 = H @ t1 -> slotA (with interleaved column sums).  Then
        # U U^T t2 = (I - e0 e0^T) t2, so
        # H-slab = e0 e0^T-slab + t2 - e0 (e0^T t2)
        #        = t2 + ones * (1 - colsum(t2))/n  (rank-1 row correction).
        HTb2 = load_full(param_block(HT_p), "HTb2")
        ps_t = pscp.tile([1, S], F32, name="ps_t")

        def emit_t2(ct, ps):
            nc.vector.tensor_copy(slotA[:, ct, :], ps[:, 0:S])
            nc.tensor.matmul(ps_t[:], ones128[:],
                             slotA[:, ct, :].bitcast(F32),
                             start=(ct == 0), stop=(ct == ET - 1))

        gemm(HTb2, lambda et: slotC[:, et, :], emit_t2)

        w2 = psc.tile([1, S], F32, name="w2")
        on1 = psc.tile([1, S], F32, name="on1")
        nc.vector.memset(on1[:], 1.0 / N)
        nc.vector.scalar_tensor_tensor(
            w2[:], ps_t[:], -1.0 / N, on1[:], op0=ALU.mult, op1=ALU.add)

        for ct in range(ET):
            ps2 = lps.tile([128, S], F32, name="psr2", tag="psr")
            nc.tensor.matmul(ps2[:], ones_r[0:1, :], w2[0:1, :],
                             start=True, stop=True)
            h1 = ltmp.tile([128, S], F32, name="h1", tag="t1")
            nc.vector.tensor_add(h1[:], slotA[:, ct, :].bitcast(F32), ps2[:])
            nc.sync.dma_start(out_p[128 * ct:128 * (ct + 1), :], h1[:])


_CACHED = {}


def _get_nc():
    if "nc" not in _CACHED:
        _CACHED["nc"] = _build_nc()
    return _CACHED["nc"]


def make_in_maps(H_raw, U):
    H_raw = np.ascontiguousarray(H_raw, np.float32)
    assert H_raw.shape == (N, N)
    Upad = np.zeros((N, N), np.float32)
    Upad[:, :U.shape[1]] = np.asarray(U, np.float32)
    HT = np.ascontiguousarray(H_raw.T)
    UT = np.ascontiguousarray(Upad.T)
    in_maps = []
    for i in range(NCORES):
        sl = slice(S * i, S * (i + 1))
        in_maps.append({
            "HTm": HT, "UTm": UT,
            "Uslab": np.ascontiguousarray(Upad[:, sl]),
            "UTslab": np.ascontiguousarray(UT[:, sl]),
        })
    return in_maps


def assemble(results):
    return np.ascontiguousarray(
        np.concatenate([results[i]["Hslab"] for i in range(NCORES)], axis=1),
        dtype=np.float32)


def kernel(H_raw, U):
    from concourse.bass_utils import run_bass_kernel_spmd
    nc = _get_nc()
    in_maps = make_in_maps(H_raw, U)
    res = run_bass_kernel_spmd(nc, in_maps, core_ids=list(range(NCORES)))
    return assemble(res.results)


if __name__ == "__main__":
    rng = np.random.default_rng(0)
    H_raw = (np.eye(N) + 0.1 / np.sqrt(N)
             * rng.standard_normal((N, N))).astype(np.float32)
    Uq, _ = np.linalg.qr(rng.standard_normal((N, N - 1)).astype(np.float32))
    out = kernel(H_raw, Uq.astype(np.float32))
    print("kernel output", out.shape, out.dtype)


# revision 26
# speedup vs baseline: 1.4181x; 1.0005x over previous
"""Trainium2 Bass kernel for nn_IsoNSProject (Newton-Schulz polar projection).

reference:  A = U^T H U  (m = n-1, padded to n=2048)
            X0 = A/sigma_max; 10 Newton-Schulz steps X <- 0.5 X (3I - X^T X)
            H_out = e0 e0^T + U X10 U^T

Device algorithm (8-core SPMD, column-slab tensor-parallel):
  All NS iterates commute (they are polynomials of C = A^T A), so the ten
  quadratic steps are replaced by an order-28 composite of two high-order
  NS steps: an order-4 step whose Gram recursion B1 = m7(B0) = B0 h7(B0)^2
  is computed as power slabs (6 chained GEMMs of resident-full^T @ slab),
  and an order-7 step that needs no successor Gram matrix, so only its
  Q-factor h13(B1) is applied -- directly to the projected slab
  z1' = Q1 @ U^T-slab, which itself is built from C-powers applied to
  U^T-slab (u-chain) in the shadow of B1's AllGather.  From the on-device
  bound c = sqrt(||C||_1) >= sigma_max(A), the scaled singular values
  start >= 0.34 and converge to 1 - ~5e-3 worst-edge (~1.8e-3 F-norm
  vs tolerance 2e-2).

  Phase 1 avoids materializing A: with G = H U and w = G^T e0,
  C = G^T G - w w^T (U U^T = I - e0 e0^T); the rank-1 term is one extra
  1-partition matmul accumulation step, and w comes from an 8KB
  AllGather of per-core column sums.  The tail exploits the projector
  again: out = t2 + ones (1 - colsum(t2))/n with t2 = H U z1, removing
  two full GEMMs.  Q's identity component q0 (tracked as a runtime
  scalar) is folded into z1' as q0 * U^T-slab.

  Collectives (5 total): AG_w (8KB), AG_G, AG_C, AllReduce(norm max),
  AG_B1 -- each 2MB-per-core slab gather; the u-chain (3 GEMMs) hides
  under AG_B1.  lhsT block loads are spread over the SP/ACT/Pool DMA
  queues; all matmuls run as float32r (full rate) with fp32 PSUM.
"""

import sys

for _p in ("/opt/trn_rl_repo", "/root/.axon_site/_ro/trn_rl_repo"):
    if _p not in sys.path:
        sys.path.insert(0, _p)

import numpy as np

import concourse.bass as bass
import concourse.tile as tile
from concourse import bacc
import concourse.mybir as mybir

N = 2048          # padded problem size (true m = 2047)
S = 256           # column-slab width per core
ET = N // 128     # 16 k-tiles
NCORES = 8

F32 = mybir.dt.float32
F32R = mybir.dt.float32r
ALU = mybir.AluOpType
AXT = mybir.AxisListType
ACT = mybir.ActivationFunctionType

# Two Newton-Schulz steps: an order-4 step X1 = X0 h7(B0) whose Gram
# recursion B1 = m7(B0) = B0 h7(B0)^2 (degree 7) is computed as power
# slabs, then an order-7 step X2 = X1 h13(B1) that needs no B2, so only
# the h13(B1) Q-factor (degree 6) is applied to the projected slab.
# From the c = sqrt(||C||_1) normalization the scaled singular values
# start >= 0.34 and reach 1 - 5e-3 (worst edge; ~1.5e-3 in F-norm)
# -- an order-28 composite, vs tolerance 2e-2.
H7 = [35.0 / 16, -35.0 / 16, 21.0 / 16, -5.0 / 16]
M7 = {j + 1: c for j, c in enumerate([
    4.78515625, -9.5703125, 10.52734375, -7.109375, 3.08984375,
    -0.8203125, 0.09765625])}
H13 = [2.9326171875, -5.865234375, 8.7978515625, -8.37890625,
       4.8876953125, -1.599609375, 0.2255859375]


def _build_nc():
    nc = bacc.Bacc(None, target_bir_lowering=False)

    HT_p = nc.declare_dram_parameter("HTm", [N, N], F32, isOutput=False)
    UT_p = nc.declare_dram_parameter("UTm", [N, N], F32, isOutput=False)
    Usl_p = nc.declare_dram_parameter("Uslab", [N, S], F32, isOutput=False)
    UTsl_p = nc.declare_dram_parameter("UTslab", [N, S], F32, isOutput=False)
    out_p = nc.declare_dram_parameter("Hslab", [N, S], F32, isOutput=True)

    RG = [list(range(NCORES))]

    with tile.TileContext(nc) as tc:
        with tc.tile_pool(name="dram", bufs=1, space="DRAM") as dram:
            bounceG = dram.tile([N, S], F32, name="bounceG")
            G_G = dram.tile([N * NCORES, S], F32, name="G_G")
            bounceW = dram.tile([1, S], F32, name="bounceW")
            G_W = dram.tile([1, NCORES * S], F32, name="G_W")
            bounceC = dram.tile([N, S], F32, name="bounceC")
            G_C = dram.tile([N * NCORES, S], F32, name="G_C")
            bounceB1 = dram.tile([N, S], F32, name="bounceB1")
            G_B2 = dram.tile([N * NCORES, S], F32, name="G_B2")
            bounceN = dram.tile([1, S], F32, name="bounceN")
            outN = dram.tile([1, S], F32, name="outN")

            def ag(in_t, out_t):
                nc.gpsimd.collective_compute(
                    "AllGather", ALU.bypass, replica_groups=RG,
                    ins=[in_t[:].opt()], outs=[out_t[:].opt()],
                )

            def param_block(p, col_off=0):
                def src(j):
                    return (p[:, col_off + S * j:col_off + S * (j + 1)]
                            .rearrange("(t p) d -> p t d", p=128).bitcast(F32R))
                return src

            def gathered_block(g, col_off=0):
                def src(j):
                    return (g[N * j:N * (j + 1), col_off:col_off + S]
                            .rearrange("(t p) d -> p t d", p=128).bitcast(F32R))
                return src

            body(tc, nc, locals())

    nc.compile()
    return nc


def body(tc, nc, T):
    HT_p, UT_p = T["HT_p"], T["UT_p"]
    Usl_p, UTsl_p, out_p = T["Usl_p"], T["UTsl_p"], T["out_p"]
    bounceG, G_G, bounceW, G_W = T["bounceG"], T["G_G"], T["bounceW"], T["G_W"]
    bounceC, G_C = T["bounceC"], T["G_C"]
    bounceB1, G_B2 = T["bounceB1"], T["G_B2"]
    bounceN, outN = T["bounceN"], T["outN"]
    ag, param_block, gathered_block = T["ag"], T["param_block"], T["gathered_block"]
    RG = [list(range(NCORES))]

    with (
        tc.tile_pool(name="lhs", bufs=1) as lhs,
        tc.tile_pool(name="lps", bufs=4, space="PSUM") as lps,
        tc.tile_pool(name="ltmp", bufs=2) as ltmp,
        tc.tile_pool(name="slab", bufs=1) as slab,
        tc.tile_pool(name="psc", bufs=1) as psc,
        tc.tile_pool(name="pscp", bufs=1, space="PSUM") as pscp,
    ):
        # lhsT block loads round-robin across three engine DMA queues so a
        # full-matrix load takes ~3 block-times instead of 8 (the cost of a
        # dma_start is charged to the issuing engine's queue).
        dma_engines = [nc.sync, nc.scalar, nc.gpsimd]

        def load_full(src, tagp):
            blks = []
            for j in range(NCORES):
                t = lhs.tile([128, ET, S], F32R, name=f"{tagp}{j}", tag=f"L{j}")
                dma_engines[j % len(dma_engines)].dma_start(t[:], src(j))
                blks.append(t)
            return blks

        def gemm(blocks, rhs_of_et, emit_out, nfree=S, extra_acc=None):
            """out[ct] = sum_et lhsT(et,ct).T @ rhs(et) (+ optional extra
            accumulation step issued with stop=True)."""
            for ct in range(ET):
                ps = lps.tile([128, nfree], F32, name="psr", tag="psr")
                j, h = ct // 2, ct % 2
                for et in range(ET):
                    nc.tensor.matmul(
                        ps[:, 0:nfree],
                        blocks[j][:, et, 128 * h:128 * (h + 1)],
                        rhs_of_et(et),
                        start=(et == 0),
                        stop=(et == ET - 1 and extra_acc is None),
                    )
                if extra_acc is not None:
                    extra_acc(ct, ps)
                emit_out(ct, ps)

        # four persistent slab slots, reused/retagged through the phases
        slotA = slab.tile([128, ET, S], F32R, name="slotA", tag="sA")
        slotB = slab.tile([128, ET, S], F32R, name="slotB", tag="sB")
        slotC = slab.tile([128, ET, S], F32R, name="slotC", tag="sC")
        slotD = slab.tile([128, ET, S], F32R, name="slotD", tag="sD")

        ones128 = psc.tile([128, 1], F32, name="ones128")
        nc.vector.memset(ones128[:], 1.0)
        ones_r = psc.tile([1, 128], F32, name="ones_r")
        nc.vector.memset(ones_r[:], 1.0)
        ws = psc.tile([1, S], F32, name="ws")
        w_neg = psc.tile([1, N], F32, name="w_neg")
        colsum = psc.tile([1, S], F32, name="colsum")
        colg = psc.tile([1, S], F32, name="colg")
        m11 = psc.tile([1, 1], F32, name="m11")

        # runtime scalars: rr^j = r^{2j} (j=1..7) at sc[:,j-1];
        # a_j = m7_j rr^j at sc[:,8+j]; q0 at sc[:,18];
        # e_j = h7_j r rr^j (j=1..3) at sc[:,18+j]; r at sc[:,23]
        sc = psc.tile([128, 24], F32, name="sc")

        def rrj(j):
            return sc[:, j - 1:j]

        def aj(j):
            return sc[:, 8 + j:9 + j]

        s_ap = sc[:, 18:19]

        def qj(j):
            return sc[:, 18 + j:19 + j]

        # ============ phase 1: G = H U (slotB), G' = H^T U, w ============
        # Uslab lives in slotD during phase 1 (freed by DS1's Bacc' writes)
        Uslab_sb = slotD
        nc.sync.dma_start(
            Uslab_sb[:],
            Usl_p.rearrange("(t p) d -> p t d", p=128).bitcast(F32R))

        ps_ws = pscp.tile([1, S], F32, name="ps_ws")
        HTb = load_full(param_block(HT_p), "HTb")

        def emit_g(ct, ps):
            nc.vector.tensor_copy(slotB[:, ct, :], ps[:, 0:S])
            nc.tensor.matmul(ps_ws[:], ones128[:],
                             slotB[:, ct, :].bitcast(F32),
                             start=(ct == 0), stop=(ct == ET - 1))
            nc.sync.dma_start(
                bounceG[128 * ct:128 * (ct + 1), :],
                slotB[:, ct, :].bitcast(F32))

        gemm(HTb, lambda et: Uslab_sb[:, et, :], emit_g)
        # ws = +colsum(G)/sqrt(n): own slab of w = G^T e0
        nc.vector.tensor_scalar_mul(ws[:], ps_ws[:], float(1.0 / np.sqrt(N)))
        nc.sync.dma_start(bounceW[:], ws[:])
        ag(bounceW, G_W)
        ag(bounceG, G_G)

        # ============ C slab (slotA) = G^T G - w w^T, norm ============
        nc.sync.dma_start(w_neg[:], G_W[:])
        nc.vector.tensor_scalar_mul(w_neg[:], w_neg[:], -1.0)

        Gb = load_full(gathered_block(G_G, 0), "Gb")
        ps_cs = pscp.tile([1, S], F32, name="ps_cs")

        def rank1_acc(ct, ps):
            nc.tensor.matmul(
                ps[:, 0:S], w_neg[0:1, 128 * ct:128 * (ct + 1)], ws[0:1, :],
                start=False, stop=True)

        def emit_c(ct, ps):
            nc.vector.tensor_copy(slotA[:, ct, :], ps[:, 0:S])
            ab = ltmp.tile([128, S], F32, name="absr", tag="t1")
            nc.vector.scalar_tensor_tensor(
                ab[:], slotA[:, ct, :].bitcast(F32), -1.0,
                slotA[:, ct, :].bitcast(F32),
                op0=ALU.mult, op1=ALU.max)
            nc.tensor.matmul(ps_cs[:], ones128[:], ab[:],
                             start=(ct == 0), stop=(ct == ET - 1))
            nc.sync.dma_start(
                bounceC[128 * ct:128 * (ct + 1), :],
                slotA[:, ct, :].bitcast(F32))

        gemm(Gb, lambda et: slotB[:, et, :], emit_c, extra_acc=rank1_acc)

        ag(bounceC, G_C)

        # ---- ||C||_1 bound -> runtime scalars ----
        nc.vector.tensor_copy(colsum[:], ps_cs[:])
        nc.sync.dma_start(bounceN[:], colsum[:])
        nc.gpsimd.collective_compute(
            "AllReduce", ALU.max, replica_groups=RG,
            ins=[bounceN[:].opt()], outs=[outN[:].opt()])
        nc.sync.dma_start(colg[:], outN[:])
        nc.vector.tensor_reduce(m11[:], colg[:], axis=AXT.X, op=ALU.max)
        ps_b = pscp.tile([128, 1], F32, name="ps_b")
        nc.tensor.matmul(ps_b[:], ones_r[:], m11[:], start=True, stop=True)

        # rr = 1/c^2 ; rr^j chain ; r = sqrt(rr) ; a_j = m7_j rr^j ;
        # q0 = h7_0 r ; e_j = h7_j r rr^j
        nc.vector.tensor_copy(rrj(1), ps_b[:])
        nc.vector.reciprocal(rrj(1), rrj(1))
        for j in range(2, 8):
            nc.vector.tensor_mul(rrj(j), rrj(j - 1), rrj(1))
        for j in range(2, 8):
            nc.vector.tensor_scalar_mul(aj(j), rrj(j), float(M7[j]))
        r_tmp = sc[:, 23:24]
        nc.scalar.activation(r_tmp, rrj(1), ACT.Sqrt)
        for j in range(1, 4):
            nc.vector.tensor_mul(qj(j), rrj(j), r_tmp)
            nc.vector.tensor_scalar_mul(qj(j), qj(j), float(H7[j]))
        nc.vector.tensor_scalar_mul(s_ap, r_tmp, float(H7[0]))
        # a_1 = m7_1 * rr
        nc.vector.tensor_scalar_mul(aj(1), rrj(1), float(M7[1]))

        # ============ DS1 (steps 0,1): powers of C ============
        # slots: A = Cs (-> Qacc), B = P-rot/B2s, C = P-rot/W-rot, D = Bacc'
        Cb = load_full(gathered_block(G_C, 0), "Cb")

        def power_chain(blocks, first_rhs, pslots, coeffs, acc, maxj=7):
            """P_{j+1} = M @ P_j for j=2..maxj; acc = sum_j coeffs[j]*P_j
            (incremental, initialized at the first present j).  coeffs[j]
            is an AP (runtime) or float immediate."""
            rhs = first_rhs
            init_j = min(coeffs)
            for j in range(2, maxj + 1):
                dst = pslots[(j - 2) % len(pslots)]

                def emit_p(ct, ps, j=j, dst=dst):
                    if j < maxj:
                        nc.vector.tensor_copy(dst[:, ct, :], ps[:, 0:S])
                    if j not in coeffs:
                        return
                    if j == init_j:
                        nc.vector.tensor_scalar_mul(
                            acc[:, ct, :], ps[:, 0:S], coeffs[j])
                    else:
                        nc.vector.scalar_tensor_tensor(
                            acc[:, ct, :], ps[:, 0:S], coeffs[j],
                            acc[:, ct, :].bitcast(F32),
                            op0=ALU.mult, op1=ALU.add)

                gemm(blocks, (lambda r: lambda et: r[:, et, :])(rhs), emit_p)
                if j < maxj:
                    rhs = dst

        # powers P2..P7 of C; Bacc' (slotD) = sum_{j=2..7} a_j C^j s
        power_chain(Cb, slotA, [slotB, slotC],
                    {j: aj(j) for j in range(2, 8)}, slotD, maxj=7)

        # bounce B2s = a1*Cs + Bacc' into slotB (P8's old slot), then AG
        for ct in range(ET):
            nc.vector.scalar_tensor_tensor(
                slotB[:, ct, :], slotA[:, ct, :].bitcast(F32), aj(1),
                slotD[:, ct, :].bitcast(F32), op0=ALU.mult, op1=ALU.add)
        nc.sync.dma_start(
            bounceB1[:].rearrange("(t p) d -> p t d", p=128),
            slotB[:].bitcast(F32))
        ag(bounceB1, G_B2)

        # ====== u-chain (shadow of AG_B1): z1' = Q_1 @ UTslab computed
        # directly from C-powers applied to the UTslab vector-slab:
        # z1' = q0*UTslab + sum_{j=1..4} e_j (C^j @ UTslab).
        # u_j -> rot(B, C); z1' accumulates in slotA (Cs is dead).
        UTslab2 = slotD
        nc.sync.dma_start(
            UTslab2[:],
            UTsl_p.rearrange("(t p) d -> p t d", p=128).bitcast(F32R))

        def emit_u1(ct, ps):
            nc.vector.tensor_copy(slotB[:, ct, :], ps[:, 0:S])
            nc.vector.scalar_tensor_tensor(
                slotA[:, ct, :], ps[:, 0:S], qj(1),
                slotA[:, ct, :].bitcast(F32), op0=ALU.mult, op1=ALU.add)

        # init A = q0 * UTslab first (DVE, before the u-chain)
        for ct in range(ET):
            nc.vector.tensor_scalar_mul(
                slotA[:, ct, :], UTslab2[:, ct, :].bitcast(F32), s_ap)

        gemm(Cb, lambda et: UTslab2[:, et, :], emit_u1)

        rhs = slotB
        for j in range(2, 4):
            dst = [slotC, slotB][j % 2]

            def emit_uj(ct, ps, j=j, dst=dst):
                if j < 3:
                    nc.vector.tensor_copy(dst[:, ct, :], ps[:, 0:S])
                nc.vector.scalar_tensor_tensor(
                    slotA[:, ct, :], ps[:, 0:S], qj(j),
                    slotA[:, ct, :].bitcast(F32), op0=ALU.mult, op1=ALU.add)

            gemm(Cb, (lambda r: lambda et: r[:, et, :])(rhs), emit_uj)
            if j < 3:
                rhs = dst

        # ====== v-chain: z1 = h13(B1) @ z1' (order-7 step applied to the
        # projected slab); v_1 = B1@z1', v_{j+1} = B1@v_j, and
        # z1 (slotB) = h13_0 z1' + sum_{j=1..6} h13_j v_j.
        B1b = load_full(gathered_block(G_B2, 0), "B1b")

        def emit_v1(ct, ps):
            nc.vector.tensor_copy(slotC[:, ct, :], ps[:, 0:S])
            nc.vector.tensor_scalar_mul(
                slotB[:, ct, :], slotA[:, ct, :].bitcast(F32), float(H13[0]))
            nc.vector.scalar_tensor_tensor(
                slotB[:, ct, :], ps[:, 0:S], float(H13[1]),
                slotB[:, ct, :].bitcast(F32), op0=ALU.mult, op1=ALU.add)

        gemm(B1b, lambda et: slotA[:, et, :], emit_v1)
        rhs = slotC
        for j in range(2, 7):
            dst = [slotD, slotC][j % 2]

            def emit_vj(ct, ps, j=j, dst=dst):
                if j < 6:
                    nc.vector.tensor_copy(dst[:, ct, :], ps[:, 0:S])
                nc.vector.scalar_tensor_tensor(
                    slotB[:, ct, :], ps[:, 0:S], float(H13[j]),
                    slotB[:, ct, :].bitcast(F32), op0=ALU.mult, op1=ALU.add)

            gemm(B1b, (lambda r: lambda et: r[:, et, :])(rhs), emit_vj)
            if j < 6:
                rhs = dst

        # t1 = U @ z1 -> slotC
        UTb = load_full(param_block(UT_p), "UTb")

        def emit_t1(ct, ps):
            nc.vector.tensor_copy(slotC[:, ct, :], ps[:, 0:S])

        gemm(UTb, lambda et: slotB[:, et, :], emit_t1)

        # t2 = H @ t1 -> slotA (with interleaved column sums).  Then
        # U U^T t2 = (I - e0 e0^T) t2, so
        # H-slab = e0 e0^T-slab + t2 - e0 (e0^T t2)
        #        = t2 + ones * (1 - colsum(t2))/n  (rank-1 row correction).
        HTb2 = load_full(param_block(HT_p), "HTb2")
        ps_t = pscp.tile([1, S], F32, name="ps_t")

        def emit_t2(ct, ps):
            nc.vector.tensor_copy(slotA[:, ct, :], ps[:, 0:S])
            nc.tensor.matmul(ps_t[:], ones128[:],
                             slotA[:, ct, :].bitcast(F32),
                             start=(ct == 0), stop=(ct == ET - 1))

        gemm(HTb2, lambda et: slotC[:, et, :], emit_t2)

        w2 = psc.tile([1, S], F32, name="w2")
        on1 = psc.tile([1, S], F32, name="on1")
        nc.vector.memset(on1[:], 1.0 / N)
        nc.vector.scalar_tensor_tensor(
            w2[:], ps_t[:], -1.0 / N, on1[:], op0=ALU.mult, op1=ALU.add)

        for ct in range(ET):
            ps2 = lps.tile([128, S], F32, name="psr2", tag="psr")
            nc.tensor.matmul(ps2[:], ones_r[0:1, :], w2[0:1, :],
                             start=True, stop=True)
            h1 = ltmp.tile([128, S], F32, name="h1", tag="t1")
            nc.vector.tensor_add(h1[:], slotA[:, ct, :].bitcast(F32), ps2[:])
            nc.sync.dma_start(out_p[128 * ct:128 * (ct + 1), :], h1[:])


_CACHED = {}


def _get_nc():
    if "nc" not in _CACHED:
        _CACHED["nc"] = _build_nc()
    return _CACHED["nc"]


def make_in_maps(H_raw, U):
    H_raw = np.ascontiguousarray(H_raw, np.float32)
    assert H_raw.shape == (N, N)
    Upad = np.zeros((N, N), np.float32)
    Upad[:, :U.shape[1]] = np.asarray(U, np.float32)
    HT = np.ascontiguousarray(H_raw.T)
    UT = np.ascontiguousarray(Upad.T)
    in_maps = []
    for i in range(NCORES):
        sl = slice(S * i, S * (i + 1))
        in_maps.append({
            "HTm": HT, "UTm": UT,
            "Uslab": np.ascontiguousarray(Upad[:, sl]),
            "UTslab": np.ascontiguousarray(UT[:, sl]),
        })
    return in_maps


def assemble(results):
    return np.ascontiguousarray(
        np.concatenate([results[i]["Hslab"] for i in range(NCORES)], axis=1),
        dtype=np.float32)


def kernel(H_raw, U):
    from concourse.bass_utils import run_bass_kernel_spmd
    nc = _get_nc()
    in_maps = make_in_maps(H_raw, U)
    res = run_bass_kernel_spmd(nc, in_maps, core_ids=list(range(NCORES)))
    return assemble(res.results)


if __name__ == "__main__":
    # smoke test; U must be the orthogonal complement of e0 = 1/sqrt(n)
    # (as the reference constructs it) for the C = G^T G - w w^T identity.
    rng = np.random.default_rng(0)
    H_raw = (np.eye(N) + 0.1 / np.sqrt(N)
             * rng.standard_normal((N, N))).astype(np.float32)
    e0 = np.ones((N, 1), np.float32) / np.sqrt(N)
    M = np.concatenate([e0, np.eye(N, dtype=np.float32)[:, 1:]], axis=1)
    Q, _ = np.linalg.qr(M)
    out = kernel(H_raw, Q[:, 1:].astype(np.float32))
    print("kernel output", out.shape, out.dtype)
